# revision 9
# baseline (speedup 1.0000x reference)
"""AnswerHead kernel for 8 TRN2 NeuronCores.

reference:  VC = VE @ W.T + b ; out[l,b,t,v] = einsum('lbtd,vd->lbtv', A, VC)

Reassociated:  logits = (A @ W) @ VE.T + (A @ b)[:, None]
  - cuts FLOPs from ~65G to ~30G (contract A with W first: A is [640, D],
    not [V, D])
  - V is sharded across the 8 cores (tensor parallel over vocab logits),
    A/W/b replicated; each core emits a [640, V/8] logit slab, host concat.

Device work per core:
  warmup : a few matmuls on a zeroed tile so the PE HAM clock-gate is at
           8/8 before real work arrives (no input dependency)
  phase 1: T^T[k, n] = sum_d W[d, k] * A^T[d, n]           (PE, 72 matmuls)
           ab[n]     = sum_d A[n, d] * b[d]                (PE, 30 matmuls)
  phase 2: out[n, v] = sum_k T^T[k, n] * VET[k, v] + ab[n] (PE, 300 matmuls,
           bias fused into the PSUM->SBUF copy on VectorE)

All host work is layout-only (transpose / cast / slice) — every FLOP is on
device.  Inputs are pre-shuffled on host into partition-major SBUF images so
each DMA descriptor is a multi-KB contiguous run (thin descriptors were the
v1 bottleneck).  Inputs stream on the sync HWDGE ring, outputs on the
scalar (ACT) HWDGE ring so reads and writes don't share a FIFO.
Compute dtype bf16 (PE runs fp32 at 1/4 rate; rel-err gate is 2e-2).
"""

import sys

if "/opt/trn_rl_repo" not in sys.path:
    sys.path.insert(0, "/opt/trn_rl_repo")

import numpy as np
import ml_dtypes

L, B, T, D, V = 2, 16, 20, 768, 30000
N = L * B * T            # 640 tokens
NCORES = 8
VS = V // NCORES         # 3750 vocab rows per core
P = 128
DC = D // P              # 6 contraction chunks of 128
NCH = N // P             # 5 token chunks of 128
G = 10                   # v groups per core
VG = VS // G             # 375 logits per group (fits one PSUM bank in f32)
N_WARM = 10              # warmup matmuls: bridge PE activity until inputs land

BF16 = ml_dtypes.bfloat16

_TRACE = False
_TRACE_KW = {}
LAST = {}
_cache = {}


def _build():
    import concourse.mybir as mybir
    import concourse.tile as tile
    from concourse import bacc

    nc = bacc.Bacc(
        "TRN2", target_bir_lowering=False, debug=False, num_devices=NCORES
    )
    bf = mybir.dt.bfloat16
    f32 = mybir.dt.float32
    add = mybir.AluOpType.add

    at_d = nc.declare_dram_parameter("at", [P, DC, N], bf, isOutput=False)
    w_d = nc.declare_dram_parameter("w", [P, DC, D], bf, isOutput=False)
    b_d = nc.declare_dram_parameter("bvec", [P, DC], bf, isOutput=False)
    vet_d = nc.declare_dram_parameter("vet", [P, G, DC, VG], bf, isOutput=False)
    out_d = nc.declare_dram_parameter("out", [G, P, NCH, VG], bf, isOutput=True)

    with tile.TileContext(nc) as tc:
        with (
            tc.tile_pool(name="const", bufs=1) as cpool,
            tc.tile_pool(name="outp", bufs=6) as opool,
            tc.tile_pool(name="ps1", bufs=2, space="PSUM") as ps1,
            tc.tile_pool(name="ps2", bufs=6, space="PSUM") as ps2,
        ):
            at_sb = cpool.tile([P, DC, N], bf, tag="at")
            w_sb = cpool.tile([P, DC, D], bf, tag="w")
            b_sb = cpool.tile([P, DC], bf, tag="b")
            vet_sb = cpool.tile([P, G, DC, VG], bf, tag="vet")
            tt_sb = cpool.tile([P, DC, N], bf, tag="tt")
            ab_sb = cpool.tile([P, NCH], f32, tag="ab")
            warm_sb = cpool.tile([P, 640], bf, tag="warm")

            # ---- PE warmup: data-independent matmuls to lift the HAM
            # clock gate to 8/8 while input DMAs are in flight.
            nc.vector.memset(warm_sb[:], 0.0)
            for i in range(N_WARM):
                wps = ps1.tile([P, 512], f32, tag="ps1")
                nc.tensor.matmul(
                    wps[:], warm_sb[:, :P], warm_sb[:, P : P + 512]
                )

            # ---- input DMAs (sync ring), critical-path first
            nc.sync.dma_start(b_sb[:], b_d.ap())
            for dc in range(DC):
                nc.sync.dma_start(at_sb[:, dc, :], at_d.ap()[:, dc, :])
                nc.sync.dma_start(w_sb[:, dc, :], w_d.ap()[:, dc, :])
            for g in range(G):
                nc.sync.dma_start(vet_sb[:, g], vet_d.ap()[:, g])

            # ---- phase 1b: T^T[k, n]  (k on partitions, per 128-chunk)
            for kc in range(DC):
                for n0, nl in ((0, 512), (512, N - 512)):
                    ps = ps1.tile([P, 512], f32, tag="ps1")
                    for dc in range(DC):
                        nc.tensor.matmul(
                            ps[:, :nl],
                            w_sb[:, dc, kc * P : (kc + 1) * P],
                            at_sb[:, dc, n0 : n0 + nl],
                            start=(dc == 0),
                            stop=(dc == DC - 1),
                        )
                    nc.vector.tensor_copy(tt_sb[:, kc, n0 : n0 + nl], ps[:, :nl])

            # ---- phase 1a: ab[n] = sum_d A[n,d] b[d], laid out [128, NCH]
            for ni in range(NCH):
                ps = ps1.tile([P, 512], f32, tag="ps1")
                for dc in range(DC):
                    nc.tensor.matmul(
                        ps[:, :1],
                        at_sb[:, dc, ni * P : (ni + 1) * P],
                        b_sb[:, dc, None],
                        start=(dc == 0),
                        stop=(dc == DC - 1),
                    )
                nc.vector.tensor_copy(ab_sb[:, ni : ni + 1], ps[:, :1])

            # ---- phase 2: logits[n, v] = T^T.T @ VET + ab
            for g in range(G):
                ot = opool.tile([P, NCH, VG], bf, tag="ot")
                for ni in range(NCH):
                    ps = ps2.tile([P, 512], f32, tag="ps2")
                    for kc in range(DC):
                        nc.tensor.matmul(
                            ps[:, :VG],
                            tt_sb[:, kc, ni * P : (ni + 1) * P],
                            vet_sb[:, g, kc, :],
                            start=(kc == 0),
                            stop=(kc == DC - 1),
                        )
                    # out = psum + ab (per-partition bias) on VectorE, cast bf16
                    nc.vector.tensor_tensor(
                        ot[:, ni, :],
                        ps[:, :VG],
                        ab_sb[:, ni, None].to_broadcast((P, VG)),
                        add,
                    )
                # one fat out DMA per group on the ACT HWDGE ring
                nc.scalar.dma_start(out_d.ap()[g], ot[:])

    nc.compile()
    return nc


def _get_nc():
    if "nc" not in _cache:
        _cache["nc"] = _build()
    return _cache["nc"]


def kernel(answer_embed, vocab_embed, W, b):
    from concourse.bass_utils import run_bass_kernel_spmd

    answer_embed = np.asarray(answer_embed, dtype=np.float32)
    vocab_embed = np.asarray(vocab_embed, dtype=np.float32)
    W = np.asarray(W, dtype=np.float32)
    b = np.asarray(b, dtype=np.float32)

    A = answer_embed.reshape(N, D)
    # partition-major images: index [p, c, ...] maps to dim value c*128+p
    at = A.reshape(N, DC, P).transpose(2, 1, 0).astype(BF16)       # [P,DC,N]
    w = W.reshape(DC, P, D).transpose(1, 0, 2).astype(BF16)        # [P,DC,D]
    bv = b.reshape(DC, P).T.astype(BF16)                           # [P,DC]

    in_maps = []
    for i in range(NCORES):
        ve_i = vocab_embed[i * VS : (i + 1) * VS]                  # [VS, D]
        vet = ve_i.reshape(G, VG, DC, P).transpose(3, 0, 2, 1).astype(BF16)
        in_maps.append({"at": at, "w": w, "bvec": bv, "vet": vet})

    nc = _get_nc()
    res = run_bass_kernel_spmd(
        nc, in_maps, core_ids=list(range(NCORES)), **(_TRACE_KW if _TRACE else {})
    )
    if _TRACE:
        LAST["exec_time_ns"] = res.exec_time_ns
        LAST["results"] = res

    # out[g, p, ni, v] -> logits[ni*128+p, g*VG+v]
    slabs = [
        res.results[i]["out"].astype(np.float32).transpose(2, 1, 0, 3).reshape(N, VS)
        for i in range(NCORES)
    ]
    full = np.concatenate(slabs, axis=1)
    return full.reshape(L, B, T, V).astype(np.float32)


# revision 10
# speedup vs baseline: 1.0243x; 1.0243x over previous
"""AnswerHead kernel for 8 TRN2 NeuronCores.

reference:  VC = VE @ W.T + b ; out[l,b,t,v] = einsum('lbtd,vd->lbtv', A, VC)

Reassociated:  logits = (A @ W) @ VE.T + (A @ b)[:, None]
  - cuts FLOPs from ~65G to ~30G (contract A with W first: A is [640, D],
    not [V, D])
  - V is sharded across the 8 cores (tensor parallel over vocab logits),
    A/W/b replicated; each core emits a [640, V/8] logit slab, host concat.

Device work per core:
  warmup : a few matmuls on a zeroed tile so the PE HAM clock-gate is at
           8/8 before real work arrives (no input dependency)
  phase 1: T^T[k, n] = sum_d W[d, k] * A^T[d, n]           (PE, 72 matmuls)
           ab[n]     = sum_d A[n, d] * b[d]                (PE, 30 matmuls)
  phase 2: out[n, v] = sum_k T^T[k, n] * VET[k, v] + ab[n] (PE, 300 matmuls,
           bias fused into the PSUM->SBUF copy on VectorE)

All host work is layout-only (transpose / cast / slice) — every FLOP is on
device.  Inputs are pre-shuffled on host into partition-major SBUF images so
each DMA descriptor is a multi-KB contiguous run (thin descriptors were the
v1 bottleneck).  Inputs stream on the sync HWDGE ring, outputs on the
scalar (ACT) HWDGE ring so reads and writes don't share a FIFO.
Compute dtype bf16 (PE runs fp32 at 1/4 rate; rel-err gate is 2e-2).
"""

import sys

if "/opt/trn_rl_repo" not in sys.path:
    sys.path.insert(0, "/opt/trn_rl_repo")

import numpy as np
import ml_dtypes

L, B, T, D, V = 2, 16, 20, 768, 30000
N = L * B * T            # 640 tokens
NCORES = 8
VS = V // NCORES         # 3750 vocab rows per core
P = 128
DC = D // P              # 6 contraction chunks of 128
NCH = N // P             # 5 token chunks of 128
G = 10                   # v groups per core
VG = VS // G             # 375 logits per group (fits one PSUM bank in f32)
N_WARM = 10              # warmup matmuls: bridge PE activity until inputs land

BF16 = ml_dtypes.bfloat16

_TRACE = False
_TRACE_KW = {}
LAST = {}
_cache = {}


def _build():
    import concourse.mybir as mybir
    import concourse.tile as tile
    from concourse import bacc

    nc = bacc.Bacc(
        "TRN2", target_bir_lowering=False, debug=False, num_devices=NCORES
    )
    bf = mybir.dt.bfloat16
    f32 = mybir.dt.float32
    add = mybir.AluOpType.add

    at_d = nc.declare_dram_parameter("at", [P, DC, N], bf, isOutput=False)
    w_d = nc.declare_dram_parameter("w", [P, DC, D], bf, isOutput=False)
    b_d = nc.declare_dram_parameter("bvec", [P, DC], bf, isOutput=False)
    vet_d = nc.declare_dram_parameter("vet", [P, G, DC, VG], bf, isOutput=False)
    out_d = nc.declare_dram_parameter("out", [G, P, NCH, VG], bf, isOutput=True)

    with tile.TileContext(nc) as tc:
        with (
            tc.tile_pool(name="const", bufs=1) as cpool,
            tc.tile_pool(name="outp", bufs=6) as opool,
            tc.tile_pool(name="ps1", bufs=2, space="PSUM") as ps1,
            tc.tile_pool(name="ps2", bufs=6, space="PSUM") as ps2,
        ):
            at_sb = cpool.tile([P, DC, N], bf, tag="at")
            w_sb = cpool.tile([P, DC, D], bf, tag="w")
            b_sb = cpool.tile([P, DC], bf, tag="b")
            vet_sb = cpool.tile([P, G, DC, VG], bf, tag="vet")
            tt_sb = cpool.tile([P, DC, N], bf, tag="tt")
            ab_sb = cpool.tile([P, NCH], f32, tag="ab")
            warm_sb = cpool.tile([P, 640], bf, tag="warm")

            # ---- PE warmup: data-independent matmuls to lift the HAM
            # clock gate to 8/8 while input DMAs are in flight.
            nc.vector.memset(warm_sb[:], 0.0)
            for i in range(N_WARM):
                wps = ps1.tile([P, 512], f32, tag="ps1")
                nc.tensor.matmul(
                    wps[:], warm_sb[:, :P], warm_sb[:, P : P + 512]
                )

            # ---- input DMAs split across BOTH HWDGE rings (the ACT ring is
            # otherwise idle until outputs start): at + even vet groups on
            # sync, w + b + odd vet groups on ACT. Critical path (at, w) first.
            for dc in range(DC):
                nc.sync.dma_start(at_sb[:, dc, :], at_d.ap()[:, dc, :])
                nc.scalar.dma_start(w_sb[:, dc, :], w_d.ap()[:, dc, :])
            nc.scalar.dma_start(b_sb[:], b_d.ap())
            for g in range(G):
                eng = nc.sync if g % 2 == 0 else nc.scalar
                eng.dma_start(vet_sb[:, g], vet_d.ap()[:, g])

            # ---- phase 1b: T^T[k, n]  (k on partitions, per 128-chunk)
            for kc in range(DC):
                for n0, nl in ((0, 512), (512, N - 512)):
                    ps = ps1.tile([P, 512], f32, tag="ps1")
                    for dc in range(DC):
                        nc.tensor.matmul(
                            ps[:, :nl],
                            w_sb[:, dc, kc * P : (kc + 1) * P],
                            at_sb[:, dc, n0 : n0 + nl],
                            start=(dc == 0),
                            stop=(dc == DC - 1),
                        )
                    nc.vector.tensor_copy(tt_sb[:, kc, n0 : n0 + nl], ps[:, :nl])

            # ---- phase 1a: ab[n] = sum_d A[n,d] b[d], laid out [128, NCH]
            for ni in range(NCH):
                ps = ps1.tile([P, 512], f32, tag="ps1")
                for dc in range(DC):
                    nc.tensor.matmul(
                        ps[:, :1],
                        at_sb[:, dc, ni * P : (ni + 1) * P],
                        b_sb[:, dc, None],
                        start=(dc == 0),
                        stop=(dc == DC - 1),
                    )
                nc.vector.tensor_copy(ab_sb[:, ni : ni + 1], ps[:, :1])

            # ---- phase 2: logits[n, v] = T^T.T @ VET + ab
            for g in range(G):
                ot = opool.tile([P, NCH, VG], bf, tag="ot")
                for ni in range(NCH):
                    ps = ps2.tile([P, 512], f32, tag="ps2")
                    for kc in range(DC):
                        nc.tensor.matmul(
                            ps[:, :VG],
                            tt_sb[:, kc, ni * P : (ni + 1) * P],
                            vet_sb[:, g, kc, :],
                            start=(kc == 0),
                            stop=(kc == DC - 1),
                        )
                    # out = psum + ab (per-partition bias) on VectorE, cast bf16
                    nc.vector.tensor_tensor(
                        ot[:, ni, :],
                        ps[:, :VG],
                        ab_sb[:, ni, None].to_broadcast((P, VG)),
                        add,
                    )
                # one fat out DMA per group on the ACT HWDGE ring
                nc.scalar.dma_start(out_d.ap()[g], ot[:])

    nc.compile()
    return nc


def _get_nc():
    if "nc" not in _cache:
        _cache["nc"] = _build()
    return _cache["nc"]


def kernel(answer_embed, vocab_embed, W, b):
    from concourse.bass_utils import run_bass_kernel_spmd

    answer_embed = np.asarray(answer_embed, dtype=np.float32)
    vocab_embed = np.asarray(vocab_embed, dtype=np.float32)
    W = np.asarray(W, dtype=np.float32)
    b = np.asarray(b, dtype=np.float32)

    A = answer_embed.reshape(N, D)
    # partition-major images: index [p, c, ...] maps to dim value c*128+p
    at = A.reshape(N, DC, P).transpose(2, 1, 0).astype(BF16)       # [P,DC,N]
    w = W.reshape(DC, P, D).transpose(1, 0, 2).astype(BF16)        # [P,DC,D]
    bv = b.reshape(DC, P).T.astype(BF16)                           # [P,DC]

    in_maps = []
    for i in range(NCORES):
        ve_i = vocab_embed[i * VS : (i + 1) * VS]                  # [VS, D]
        vet = ve_i.reshape(G, VG, DC, P).transpose(3, 0, 2, 1).astype(BF16)
        in_maps.append({"at": at, "w": w, "bvec": bv, "vet": vet})

    nc = _get_nc()
    res = run_bass_kernel_spmd(
        nc, in_maps, core_ids=list(range(NCORES)), **(_TRACE_KW if _TRACE else {})
    )
    if _TRACE:
        LAST["exec_time_ns"] = res.exec_time_ns
        LAST["results"] = res

    # out[g, p, ni, v] -> logits[ni*128+p, g*VG+v]
    slabs = [
        res.results[i]["out"].astype(np.float32).transpose(2, 1, 0, 3).reshape(N, VS)
        for i in range(NCORES)
    ]
    full = np.concatenate(slabs, axis=1)
    return full.reshape(L, B, T, V).astype(np.float32)


# revision 12
# speedup vs baseline: 1.0827x; 1.0569x over previous
"""AnswerHead kernel for 8 TRN2 NeuronCores.

reference:  VC = VE @ W.T + b ; out[l,b,t,v] = einsum('lbtd,vd->lbtv', A, VC)

Reassociated:  logits = (A @ W) @ VE.T + (A @ b)[:, None]
  - cuts FLOPs from ~65G to ~30G (contract A with W first: A is [640, D],
    not [V, D])
  - V is sharded across the 8 cores (tensor parallel over vocab logits),
    A/W/b replicated; each core emits a [640, V/8] logit slab, host concat.

Device work per core:
  warmup : data-independent matmuls so the PE HAM clock-gate is at 8/8
           before real work arrives
  phase 1: T^T[k, n] = sum_d W[d, k] * A^T[d, n]           (PE, 72 matmuls)
           ab[n]     = sum_d A[n, d] * b[d]                (PE, 30 matmuls)
  phase 2: out[n, v] = sum_k T^T[k, n] * VET[k, v] + ab[n] (PE, 300 matmuls,
           bias fused into the PSUM->SBUF copy on VectorE)

All host work is layout-only (transpose / cast / slice) — every FLOP is on
device.  Inputs are pre-shuffled on host into partition-major SBUF images so
each DMA descriptor is a multi-KB contiguous run.  W is fed in k-major
chunks so phase 1's first kc-group depends on `at` plus only 1/6 of W — the
PE, not the DMA stream, then paces phase 1.  Inputs stream on the sync
HWDGE ring, outputs (bf16) on the scalar (ACT) HWDGE ring so reads and
writes don't share a FIFO.
Compute dtype bf16 (PE runs fp32 at 1/4 rate; rel-err gate is 2e-2).
"""

import sys

if "/opt/trn_rl_repo" not in sys.path:
    sys.path.insert(0, "/opt/trn_rl_repo")

import numpy as np
import ml_dtypes

L, B, T, D, V = 2, 16, 20, 768, 30000
N = L * B * T            # 640 tokens
NCORES = 8
VS = V // NCORES         # 3750 vocab rows per core
P = 128
DC = D // P              # 6 contraction chunks of 128
KC = D // P              # 6 output-k chunks of 128 (phase 1)
NCH = N // P             # 5 token chunks of 128
G = 10                   # v groups per core
VG = VS // G             # 375 logits per group (fits one PSUM bank in f32)
N_WARM = 14              # warmup matmuls: bridge PE activity until inputs land

BF16 = ml_dtypes.bfloat16

_TRACE = False
_TRACE_KW = {}
LAST = {}
_cache = {}


def _build():
    import concourse.mybir as mybir
    import concourse.tile as tile
    from concourse import bacc

    nc = bacc.Bacc(
        "TRN2", target_bir_lowering=False, debug=False, num_devices=NCORES
    )
    bf = mybir.dt.bfloat16
    f32 = mybir.dt.float32
    add = mybir.AluOpType.add

    at_d = nc.declare_dram_parameter("at", [P, DC, N], bf, isOutput=False)
    w_d = nc.declare_dram_parameter("w", [P, KC, DC, P], bf, isOutput=False)
    b_d = nc.declare_dram_parameter("bvec", [P, DC], bf, isOutput=False)
    vet_d = nc.declare_dram_parameter("vet", [P, G, DC, VG], bf, isOutput=False)
    out_d = nc.declare_dram_parameter("out", [G, P, NCH, VG], bf, isOutput=True)

    with tile.TileContext(nc) as tc:
        with (
            tc.tile_pool(name="const", bufs=1) as cpool,
            tc.tile_pool(name="outp", bufs=6) as opool,
            tc.tile_pool(name="ps1", bufs=2, space="PSUM") as ps1,
            tc.tile_pool(name="ps2", bufs=6, space="PSUM") as ps2,
        ):
            at_sb = cpool.tile([P, DC, N], bf, tag="at")
            w_sb = cpool.tile([P, KC, DC, P], bf, tag="w")
            b_sb = cpool.tile([P, DC], bf, tag="b")
            vet_sb = cpool.tile([P, G, DC, VG], bf, tag="vet")
            tt_sb = cpool.tile([P, KC, N], bf, tag="tt")
            ab_sb = cpool.tile([P, NCH], f32, tag="ab")
            warm_sb = cpool.tile([P, 640], bf, tag="warm")

            # ---- PE warmup: data-independent matmuls to lift the HAM
            # clock gate to 8/8 while input DMAs are in flight.
            nc.vector.memset(warm_sb[:], 0.0)
            for i in range(N_WARM):
                wps = ps1.tile([P, 512], f32, tag="ps1")
                nc.tensor.matmul(
                    wps[:], warm_sb[:, :P], warm_sb[:, P : P + 512]
                )

            # ---- input DMAs (sync ring), critical-path first:
            # at (everything needs it), then W in kc chunks (phase 1 follows
            # them), then b, then vet groups (phase 2 follows them).
            for dc in range(DC):
                nc.sync.dma_start(at_sb[:, dc, :], at_d.ap()[:, dc, :])
            for kc in range(KC):
                nc.sync.dma_start(w_sb[:, kc], w_d.ap()[:, kc])
            nc.sync.dma_start(b_sb[:], b_d.ap())
            for g in range(G):
                nc.sync.dma_start(vet_sb[:, g], vet_d.ap()[:, g])

            # ---- phase 1: T^T[k, n]  (k on partitions, per 128-chunk);
            # group kc depends on at + w[:, kc] only.
            for kc in range(KC):
                for n0, nl in ((0, 512), (512, N - 512)):
                    ps = ps1.tile([P, 512], f32, tag="ps1")
                    for dc in range(DC):
                        nc.tensor.matmul(
                            ps[:, :nl],
                            w_sb[:, kc, dc, :],
                            at_sb[:, dc, n0 : n0 + nl],
                            start=(dc == 0),
                            stop=(dc == DC - 1),
                        )
                    nc.vector.tensor_copy(tt_sb[:, kc, n0 : n0 + nl], ps[:, :nl])

            # ---- ab[n] = sum_d A[n,d] b[d], laid out [128, NCH]
            for ni in range(NCH):
                ps = ps1.tile([P, 512], f32, tag="ps1")
                for dc in range(DC):
                    nc.tensor.matmul(
                        ps[:, :1],
                        at_sb[:, dc, ni * P : (ni + 1) * P],
                        b_sb[:, dc, None],
                        start=(dc == 0),
                        stop=(dc == DC - 1),
                    )
                nc.vector.tensor_copy(ab_sb[:, ni : ni + 1], ps[:, :1])

            # ---- phase 2: logits[n, v] = T^T.T @ VET + ab
            for g in range(G):
                last_g = g == G - 1
                ot = opool.tile([P, NCH, VG], bf, tag="ot")
                for ni in range(NCH):
                    ps = ps2.tile([P, 512], f32, tag="ps2")
                    for kc in range(KC):
                        nc.tensor.matmul(
                            ps[:, :VG],
                            tt_sb[:, kc, ni * P : (ni + 1) * P],
                            vet_sb[:, g, kc, :],
                            start=(kc == 0),
                            stop=(kc == KC - 1),
                        )
                    # out = psum + ab (per-partition bias) on VectorE, cast bf16
                    nc.vector.tensor_tensor(
                        ot[:, ni, :],
                        ps[:, :VG],
                        ab_sb[:, ni, None].to_broadcast((P, VG)),
                        add,
                    )
                    if last_g:
                        # final group: ship each slice as soon as its copy
                        # lands, so the tail isn't one big trailing DMA
                        nc.scalar.dma_start(
                            out_d.ap()[g, :, ni, :], ot[:, ni, :]
                        )
                if not last_g:
                    # one fat out DMA per group on the ACT HWDGE ring
                    nc.scalar.dma_start(out_d.ap()[g], ot[:])

    nc.compile()
    return nc


def _get_nc():
    if "nc" not in _cache:
        _cache["nc"] = _build()
    return _cache["nc"]


def kernel(answer_embed, vocab_embed, W, b):
    from concourse.bass_utils import run_bass_kernel_spmd

    answer_embed = np.asarray(answer_embed, dtype=np.float32)
    vocab_embed = np.asarray(vocab_embed, dtype=np.float32)
    W = np.asarray(W, dtype=np.float32)
    b = np.asarray(b, dtype=np.float32)

    A = answer_embed.reshape(N, D)
    # partition-major images: index [p, c, ...] maps to dim value c*128+p
    at = A.reshape(N, DC, P).transpose(2, 1, 0).astype(BF16)       # [P,DC,N]
    # W image [p, kc, dc, kcol]: W[dc*128+p, kc*128+kcol]
    w = W.reshape(DC, P, KC, P).transpose(1, 2, 0, 3).astype(BF16)
    bv = b.reshape(DC, P).T.astype(BF16)                           # [P,DC]

    in_maps = []
    for i in range(NCORES):
        ve_i = vocab_embed[i * VS : (i + 1) * VS]                  # [VS, D]
        vet = ve_i.reshape(G, VG, DC, P).transpose(3, 0, 2, 1).astype(BF16)
        in_maps.append({"at": at, "w": w, "bvec": bv, "vet": vet})

    nc = _get_nc()
    res = run_bass_kernel_spmd(
        nc, in_maps, core_ids=list(range(NCORES)), **(_TRACE_KW if _TRACE else {})
    )
    if _TRACE:
        LAST["exec_time_ns"] = res.exec_time_ns
        LAST["results"] = res

    # out[g, p, ni, v] -> logits[ni*128+p, g*VG+v]
    slabs = [
        res.results[i]["out"].astype(np.float32).transpose(2, 1, 0, 3).reshape(N, VS)
        for i in range(NCORES)
    ]
    full = np.concatenate(slabs, axis=1)
    return full.reshape(L, B, T, V).astype(np.float32)
